# revision 1
# baseline (speedup 1.0000x reference)
"""Trainium2 Bass kernel for nn_AttentionFFM.

Reference computation, per token (b, k) with v = x[b, :, k] (a 64-vector)
and constant w = vk @ vk.T (64x64, symmetric):

    s_ij   = v_i * v_j
    z_ij   = s_ij * w_ij
    out_i  = (sum_j exp(z_ij) * s_ij) / (sum_j exp(z_ij))

(the v_i / v_j softmax-weighting factors are absorbed exactly by using
s inside the numerator sum; softmax max-subtraction is skipped since
|z| < ~11 for these inputs, well within fp32/bf16 exp range).

Layout (per core; batch-parallel across 8 cores, 128 batches each):
  - partitions = batch b (128), free = (i, j) for one k-slice at a time
  - the x tile [128, 1024] is x[b] contiguous; v_i and v_j enter the
    64x64 outer-product via stride tricks (i: step 16 / broadcast 0,
    j: broadcast 0 / step 16) on that one tile -- no transposes.
  - s, z, e=exp(z), q=e*s are bf16 [128, 4096]; row-sums of e and q are
    pairwise-halving trees of dense 2x-mode bf16 adds; final level,
    reciprocal and the output multiply are fp32.
  - All working tiles are allocated ONCE and ping-ponged by k parity.
    (Tile-pool slot reuse triggers a hardware fault/hang in this
    environment, so no per-iteration pool.tile() allocations.)
"""

import sys
from contextlib import ExitStack

import numpy as np

if "/opt/trn_rl_repo" not in sys.path:
    sys.path.insert(0, "/opt/trn_rl_repo")

import concourse.bass as bass
import concourse.tile as tile
from concourse import bacc, mybir
from concourse.bass_utils import run_bass_kernel_spmd

# Optional NEFF compile cache (keyed by BIR hash, traceback metadata
# stripped) — skips the multi-minute walrus compile when this exact kernel
# was compiled before on this machine. Falls back to a normal compile.
_NEFF_CACHE_DIR = "/tmp/bass_neff_cache"


def _install_neff_cache():
    import hashlib
    import shutil

    from concourse import bass_utils as _bu

    if getattr(_bu.compile_bir_kernel, "_is_cached_wrapper", False):
        return

    _orig = _bu.compile_bir_kernel

    _volatile = {"ant_traceback", "filename", "lineno", "kernel_name"}

    def _strip(obj):
        if isinstance(obj, dict):
            return {k: _strip(v) for k, v in obj.items() if k not in _volatile}
        if isinstance(obj, list):
            return [_strip(v) for v in obj]
        return obj

    def _key(bir_json):
        import orjson

        try:
            normalized = orjson.dumps(_strip(orjson.loads(bir_json)))
        except Exception:
            normalized = bir_json
        return hashlib.sha256(normalized).hexdigest()[:32]

    def _cached(bir_json, tmpdir, neff_name="file.neff"):
        import os as _os

        try:
            _os.makedirs(_NEFF_CACHE_DIR, exist_ok=True)
            p = _os.path.join(_NEFF_CACHE_DIR, _key(bir_json) + ".neff")
            dst = _os.path.join(tmpdir, neff_name)
            if _os.path.exists(p):
                shutil.copy(p, dst)
                return dst
            out = _orig(bir_json, tmpdir, neff_name)
            try:
                shutil.copy(out, p)
            except Exception:
                pass
            return out
        except Exception:
            return _orig(bir_json, tmpdir, neff_name)

    _cached._is_cached_wrapper = True
    _bu.compile_bir_kernel = _cached
    try:
        import concourse.bass2jax as _b2j

        if hasattr(_b2j, "compile_bir_kernel"):
            _b2j.compile_bir_kernel = _cached
    except Exception:
        pass


_install_neff_cache()

B, M, K = 1024, 64, 16
NCORES = 8
BL = B // NCORES  # batches per core

_CACHE = {}
LAST_RESULTS = None
TRACE = False

# Debug/bisect knobs (only for local testing; defaults = production kernel).
K_LIMIT = K
LINEARIZE = False
NBUF = 2  # parity buffers for working tiles
S_ENGINE = "vector"  # "vector" | "gpsimd" — engine for the s outer-product
X_COPY = False  # read v_j from a duplicate x tile (avoid same-tensor 2-port read)


def _tree_tiles(pool, prefix):
    """Pre-allocate the pairwise-reduction level tiles for one tensor."""
    tiles = {}
    width = M // 2
    while width >= 2:
        tiles[width] = pool.tile(
            [BL, M, width], mybir.dt.bfloat16, tag=f"{prefix}{width}",
            name=f"{prefix}{width}",
        )
        width //= 2
    tiles["res"] = pool.tile(
        [BL, M], mybir.dt.float32, tag=f"{prefix}r", name=f"{prefix}r"
    )
    return tiles


def _reduce_tree(nc, t, tiles):
    """Row-sums over j of t [BL, M, M] (bf16) -> tiles['res'] [BL, M] fp32."""
    cur = t
    width = M // 2
    while width >= 2:
        nxt = tiles[width]
        nc.vector.tensor_tensor(
            out=nxt[:, :, :],
            in0=cur[:, :, 0:width],
            in1=cur[:, :, width : 2 * width],
            op=mybir.AluOpType.add,
        )
        cur = nxt
        width //= 2
    res = tiles["res"]
    nc.vector.tensor_tensor(
        out=res[:, :],
        in0=cur[:, :, 0],
        in1=cur[:, :, 1],
        op=mybir.AluOpType.add,
    )
    return res


def _build():
    nc = bacc.Bacc(
        "TRN2",
        target_bir_lowering=False,
        debug=False,
        num_devices=NCORES,
    )
    x_in = nc.declare_dram_parameter("x", [BL, M * K], mybir.dt.float32, isOutput=False)
    w_in = nc.declare_dram_parameter(
        "w", [1, M * M], mybir.dt.bfloat16, isOutput=False
    )
    out_ext = nc.declare_dram_parameter(
        "out", [BL, M * K], mybir.dt.float32, isOutput=True
    )

    with tile.TileContext(nc, linearize=LINEARIZE) as tc, ExitStack() as ctx:
        const = ctx.enter_context(tc.tile_pool(name="const", bufs=1))
        big = ctx.enter_context(tc.tile_pool(name="big", bufs=1))
        trees = ctx.enter_context(tc.tile_pool(name="trees", bufs=1))

        x_sb = const.tile([BL, M * K], mybir.dt.float32)
        nc.sync.dma_start(out=x_sb[:, :], in_=x_in[:, :])
        if X_COPY:
            x_sb2 = const.tile([BL, M * K], mybir.dt.float32)
            nc.sync.dma_start(out=x_sb2[:, :], in_=x_in[:, :])
        else:
            x_sb2 = x_sb

        w_bf = const.tile([BL, M * M], mybir.dt.bfloat16)
        w_bcast = bass.AP(
            tensor=w_in[0:1, :].tensor,
            offset=w_in[0:1, :].offset,
            ap=[[0, BL], [1, M * M]],
        )
        nc.gpsimd.dma_start(out=w_bf[:, :], in_=w_bcast)

        out_sb = const.tile([BL, M * K], mybir.dt.float32)
        out_3d = out_sb[:, :].rearrange("p (i k) -> p i k", k=K)
        x_3d = x_sb[:, :].rearrange("p (i k) -> p i k", k=K)
        x2_3d = x_sb2[:, :].rearrange("p (i k) -> p i k", k=K)

        # Pre-allocated ping-pong working tiles (no pool slot cycling).
        s_t = [
            big.tile([BL, M, M], mybir.dt.bfloat16, tag=f"s{p}", name=f"s{p}")
            for p in range(NBUF)
        ]
        z_t = [
            big.tile([BL, M * M], mybir.dt.bfloat16, tag=f"z{p}", name=f"z{p}")
            for p in range(NBUF)
        ]
        e_t = [
            big.tile([BL, M, M], mybir.dt.bfloat16, tag=f"e{p}", name=f"e{p}")
            for p in range(NBUF)
        ]
        q_t = [
            big.tile([BL, M, M], mybir.dt.bfloat16, tag=f"q{p}", name=f"q{p}")
            for p in range(NBUF)
        ]
        dt_t = [_tree_tiles(trees, f"d{p}") for p in range(NBUF)]
        nt_t = [_tree_tiles(trees, f"n{p}") for p in range(NBUF)]
        rd_t = [
            trees.tile([BL, M], mybir.dt.float32, tag=f"rd{p}", name=f"rd{p}")
            for p in range(NBUF)
        ]

        for k in range(K_LIMIT):
            p = k % NBUF
            xk = x_3d[:, :, k]  # [BL, M] view of v (strided by K)
            xi = xk.unsqueeze(-1).broadcast_to((BL, M, M))
            xj = x2_3d[:, :, k].unsqueeze(1).broadcast_to((BL, M, M))

            s = s_t[p]
            s_eng = nc.gpsimd if S_ENGINE == "gpsimd" else nc.vector
            s_eng.tensor_tensor(
                out=s[:, :, :], in0=xi, in1=xj, op=mybir.AluOpType.mult
            )
            s_flat = s[:, :, :].rearrange("p i j -> p (i j)")

            z = z_t[p]
            nc.vector.tensor_tensor(
                out=z[:, :], in0=s_flat, in1=w_bf[:, :], op=mybir.AluOpType.mult
            )

            e = e_t[p]
            nc.scalar.activation(
                out=e[:, :, :].rearrange("p i j -> p (i j)"),
                in_=z[:, :],
                func=mybir.ActivationFunctionType.Exp,
            )

            q = q_t[p]
            nc.vector.tensor_tensor(
                out=q[:, :, :].rearrange("p i j -> p (i j)"),
                in0=e[:, :, :].rearrange("p i j -> p (i j)"),
                in1=s_flat,
                op=mybir.AluOpType.mult,
            )

            denom = _reduce_tree(nc, e, dt_t[p])
            numer = _reduce_tree(nc, q, nt_t[p])

            rdenom = rd_t[p]
            nc.vector.reciprocal(out=rdenom[:, :], in_=denom[:, :])
            nc.vector.tensor_tensor(
                out=out_3d[:, :, k],
                in0=numer[:, :],
                in1=rdenom[:, :],
                op=mybir.AluOpType.mult,
            )

        nc.sync.dma_start(out=out_ext[:, :], in_=out_sb[:, :])

    nc.compile()
    return nc


def _get_nc():
    if "nc" not in _CACHE:
        _CACHE["nc"] = _build()
    return _CACHE["nc"]


def kernel(x, vk):
    global LAST_RESULTS
    x = np.ascontiguousarray(np.asarray(x), dtype=np.float32)
    vk = np.ascontiguousarray(np.asarray(vk), dtype=np.float32)
    assert x.shape == (B, M, K) and vk.shape[0] == M

    import ml_dtypes

    w = (vk @ vk.T).astype(ml_dtypes.bfloat16).reshape(1, M * M)
    xs = x.reshape(NCORES, BL, M * K)
    in_maps = [{"x": xs[i], "w": w} for i in range(NCORES)]

    nc = _get_nc()
    res = run_bass_kernel_spmd(nc, in_maps, core_ids=list(range(NCORES)), trace=TRACE)
    LAST_RESULTS = res
    out = np.concatenate(
        [np.asarray(res.results[i]["out"]).reshape(BL, M, K) for i in range(NCORES)],
        axis=0,
    )
    return out.astype(np.float32, copy=False)



# revision 4
# speedup vs baseline: 1.8432x; 1.8432x over previous
"""Trainium2 Bass kernel for nn_AttentionFFM — j-on-partitions layout (v2).

Per token (b, k), v = x[b, :, k]:
    e_ij  = exp(w_ij v_i v_j),  out_i = v_i * (sum_j e_ij v_j) / (sum_j e_ij)

v2 layout: process a k-PAIR at a time with partition p = (k2, j)
(k2 = p//64, j = p%64) and free = (i outer 64, b inner 128):

  - The j-sums become PE matmuls: for each i, stationary = e[:, i, :]
    ([128 part, 128 b cols]), moving = a [128, 2] k2-selector; the
    [128, 2] PSUM output lands directly in (b-partition, k2-col) layout.
    This removes the reduction trees (the v1 DVE bottleneck) entirely.
  - v_j is partition-local (an inner-stride-1 view of a small [128, 8, 128]
    transposed-x tile), v_i rides a DMA partition-broadcast of a
    CPU-pretransposed flat row, w rides a CPU-prebuilt replicated tile:
    every big DVE tensor_tensor runs in 2x bf16 mode.
  - DVE per pair: s = xtj*xi_rep, z = s*w_rep, u(->s) = e*xtj, recip, o1.
  - ACT: one 8192-elem exp per pair. Pool: final v_i multiply into out.
  - All tiles preallocated, ping-ponged by pair parity (no pool cycling).
"""

import sys
from contextlib import ExitStack

import numpy as np

if "/opt/trn_rl_repo" not in sys.path:
    sys.path.insert(0, "/opt/trn_rl_repo")

import concourse.bass as bass
import concourse.tile as tile
from concourse import bacc, mybir
from concourse.bass import MemorySpace
from concourse.bass_utils import run_bass_kernel_spmd

_NEFF_CACHE_DIR = "/tmp/bass_neff_cache"


def _install_neff_cache():
    import hashlib
    import shutil

    from concourse import bass_utils as _bu

    if getattr(_bu.compile_bir_kernel, "_is_cached_wrapper", False):
        return

    _orig = _bu.compile_bir_kernel

    _volatile = {"ant_traceback", "filename", "lineno", "kernel_name"}

    def _strip(obj):
        if isinstance(obj, dict):
            return {k: _strip(v) for k, v in obj.items() if k not in _volatile}
        if isinstance(obj, list):
            return [_strip(v) for v in obj]
        return obj

    def _key(bir_json):
        import orjson

        try:
            normalized = orjson.dumps(_strip(orjson.loads(bir_json)))
        except Exception:
            normalized = bir_json
        return hashlib.sha256(normalized).hexdigest()[:32]

    def _cached(bir_json, tmpdir, neff_name="file.neff"):
        import os as _os

        try:
            _os.makedirs(_NEFF_CACHE_DIR, exist_ok=True)
            p = _os.path.join(_NEFF_CACHE_DIR, _key(bir_json) + ".neff")
            dst = _os.path.join(tmpdir, neff_name)
            if _os.path.exists(p):
                shutil.copy(p, dst)
                return dst
            out = _orig(bir_json, tmpdir, neff_name)
            try:
                shutil.copy(out, p)
            except Exception:
                pass
            return out
        except Exception:
            return _orig(bir_json, tmpdir, neff_name)

    _cached._is_cached_wrapper = True
    _bu.compile_bir_kernel = _cached
    try:
        import concourse.bass2jax as _b2j

        if hasattr(_b2j, "compile_bir_kernel"):
            _b2j.compile_bir_kernel = _cached
    except Exception:
        pass


_install_neff_cache()

B, M, K = 1024, 64, 16
NCORES = 8
BL = B // NCORES
NP = K // 2  # k-pairs

_CACHE = {}
LAST_RESULTS = None
TRACE = False
P_LIMIT = NP
LINEARIZE = False


def _build():
    nc = bacc.Bacc(
        "TRN2",
        target_bir_lowering=False,
        debug=False,
        num_devices=NCORES,
    )
    bf16 = mybir.dt.bfloat16
    f32 = mybir.dt.float32
    mult = mybir.AluOpType.mult

    x_in = nc.declare_dram_parameter("x", [BL, M * K], f32, isOutput=False)
    wrep_in = nc.declare_dram_parameter("w_rep", [128, M * BL], bf16, isOutput=False)
    xtj_in = nc.declare_dram_parameter("xtj", [128, NP * BL], bf16, isOutput=False)
    xflat_in = nc.declare_dram_parameter("xflat", [K, M * BL], bf16, isOutput=False)
    sel_in = nc.declare_dram_parameter("sel", [128, 2], bf16, isOutput=False)
    out_ext = nc.declare_dram_parameter("out", [BL, M * K], f32, isOutput=True)

    FREE = M * BL  # 8192

    with tile.TileContext(nc, linearize=LINEARIZE) as tc, ExitStack() as ctx:
        const = ctx.enter_context(tc.tile_pool(name="const", bufs=1))
        big = ctx.enter_context(tc.tile_pool(name="big", bufs=1))
        ps = ctx.enter_context(tc.tile_pool(name="ps", bufs=1, space=MemorySpace.PSUM))

        x_sb = const.tile([BL, M * K], f32)
        nc.sync.dma_start(out=x_sb[:, :], in_=x_in[:, :])
        w_rep = const.tile([128, FREE], bf16)
        nc.gpsimd.dma_start(out=w_rep[:, :], in_=wrep_in[:, :])
        xtj = const.tile([128, NP, BL], bf16)
        nc.scalar.dma_start(
            out=xtj[:, :, :], in_=xtj_in[:, :].rearrange("p (q b) -> p q b", b=BL)
        )
        sel = const.tile([128, 2], bf16)
        nc.scalar.dma_start(out=sel[:, :], in_=sel_in[:, :])
        out_sb = const.tile([BL, M * K], f32)

        xi_t = [big.tile([128, FREE], bf16, tag=f"xi{c}", name=f"xi{c}") for c in range(2)]
        s_t = [big.tile([128, FREE], bf16, tag=f"s{c}", name=f"s{c}") for c in range(2)]
        z_t = [big.tile([128, FREE], bf16, tag=f"z{c}", name=f"z{c}") for c in range(2)]
        e_t = [big.tile([128, FREE], bf16, tag=f"e{c}", name=f"e{c}") for c in range(2)]
        rd_t = [big.tile([BL, 2 * M], f32, tag=f"rd{c}", name=f"rd{c}") for c in range(2)]
        o1_t = [big.tile([BL, 2 * M], f32, tag=f"o1{c}", name=f"o1{c}") for c in range(2)]
        D_ps = [ps.tile([BL, 2 * M], f32, tag=f"D{c}", name=f"D{c}") for c in range(2)]
        T_ps = [ps.tile([BL, 2 * M], f32, tag=f"T{c}", name=f"T{c}") for c in range(2)]

        def bcast_xi(q, c):
            # xi_rep[c]: partitions [0:64) get xflat[2q], [64:128) get xflat[2q+1]
            for k2 in range(2):
                src = bass.AP(
                    tensor=xflat_in[2 * q + k2 : 2 * q + k2 + 1, :].tensor,
                    offset=xflat_in[2 * q + k2 : 2 * q + k2 + 1, :].offset,
                    ap=[[0, 64], [1, FREE]],
                )
                nc.sync.dma_start(out=xi_t[c][64 * k2 : 64 * (k2 + 1), :], in_=src)

        def xtj_view(q):  # [128, M(i), BL(b)]: value = v_j(partition) per b
            return xtj[:, q, :].unsqueeze(1).broadcast_to((128, M, BL))

        for q in range(min(2, P_LIMIT)):
            bcast_xi(q, q % 2)

        for q in range(P_LIMIT):
            c = q % 2
            s = s_t[c]
            z = z_t[c]
            e = e_t[c]
            s3 = s[:, :].rearrange("p (i b) -> p i b", b=BL)
            nc.vector.tensor_tensor(out=s3, in0=xtj_view(q), in1=xi_t[c][:, :].rearrange(
                "p (i b) -> p i b", b=BL), op=mult)
            if q + 2 < P_LIMIT:
                bcast_xi(q + 2, c)
            nc.vector.tensor_tensor(out=z[:, :], in0=s[:, :], in1=w_rep[:, :], op=mult)
            nc.scalar.activation(
                out=e[:, :], in_=z[:, :], func=mybir.ActivationFunctionType.Exp
            )
            # u = e * v_j, overwriting s (s is dead after z).
            nc.vector.tensor_tensor(
                out=s3, in0=e[:, :].rearrange("p (i b) -> p i b", b=BL),
                in1=xtj_view(q), op=mult,
            )
            e3 = e[:, :].rearrange("p (i b) -> p i b", b=BL)
            # Interleave D and T matmuls so consecutive PE ops target
            # different PSUM banks (avoids same-bank write turnaround).
            for i in range(M):
                nc.tensor.matmul(
                    D_ps[c][:, 2 * i : 2 * i + 2], e3[:, i, :], sel[:, :],
                    start=True, stop=True,
                )
                nc.tensor.matmul(
                    T_ps[c][:, 2 * i : 2 * i + 2], s3[:, i, :], sel[:, :],
                    start=True, stop=True,
                )
            rd = rd_t[c]
            nc.vector.reciprocal_approx_fast(out=rd[:, :], in_=D_ps[c][:, :])
            o1 = o1_t[c]
            nc.vector.tensor_tensor(
                out=o1[:, :], in0=T_ps[c][:, :], in1=rd[:, :], op=mult
            )
            # out[b, i, 2q + k2] = v * o1 ; o1 free layout = (i, k2)
            o1v = o1[:, :].rearrange("p (i t) -> p i t", t=2)
            xv = x_sb[:, :].rearrange("p (i k) -> p i k", k=K)[:, :, 2 * q : 2 * q + 2]
            outv = out_sb[:, :].rearrange("p (i k) -> p i k", k=K)[
                :, :, 2 * q : 2 * q + 2
            ]
            nc.gpsimd.tensor_tensor(out=outv, in0=o1v, in1=xv, op=mult)

        nc.sync.dma_start(out=out_ext[:, :], in_=out_sb[:, :])

    nc.compile()
    return nc


def _get_nc():
    if "nc" not in _CACHE:
        _CACHE["nc"] = _build()
    return _CACHE["nc"]


def _prep_core(xc, wb):
    """CPU-side layout prep for one core. xc [BL, M, K] f32, wb [M, M] bf16."""
    import ml_dtypes

    bf = ml_dtypes.bfloat16
    xb = xc.astype(bf)
    xt = xb.transpose(2, 1, 0)  # [k, j, b]
    xtj = (
        xt.reshape(NP, 2, M, BL).transpose(1, 2, 0, 3).reshape(128, NP * BL)
    )  # [(k2 j), (q b)]
    xflat = xt.reshape(K, M * BL)  # [k, (i b)] (i outer, b inner)
    wrep = np.broadcast_to(
        np.ascontiguousarray(wb)[:, :, None], (M, M, BL)
    ).reshape(M, M * BL)
    w_rep = np.concatenate([wrep, wrep], axis=0)  # [(k2 j), (i b)]
    return {
        "x": np.ascontiguousarray(xc.reshape(BL, M * K)),
        "w_rep": np.ascontiguousarray(w_rep),
        "xtj": np.ascontiguousarray(xtj),
        "xflat": np.ascontiguousarray(xflat),
    }


def kernel(x, vk):
    global LAST_RESULTS
    x = np.ascontiguousarray(np.asarray(x), dtype=np.float32)
    vk = np.ascontiguousarray(np.asarray(vk), dtype=np.float32)
    assert x.shape == (B, M, K) and vk.shape[0] == M

    import ml_dtypes

    bf = ml_dtypes.bfloat16
    wb = (vk @ vk.T).astype(bf)
    sel = np.zeros((128, 2), dtype=bf)
    sel[:64, 0] = 1
    sel[64:, 1] = 1

    in_maps = []
    for i in range(NCORES):
        m = _prep_core(x[i * BL : (i + 1) * BL], wb)
        m["sel"] = sel
        in_maps.append(m)

    nc = _get_nc()
    res = run_bass_kernel_spmd(nc, in_maps, core_ids=list(range(NCORES)), trace=TRACE)
    LAST_RESULTS = res
    out = np.concatenate(
        [np.asarray(res.results[i]["out"]).reshape(BL, M, K) for i in range(NCORES)],
        axis=0,
    )
    return out.astype(np.float32, copy=False)
